# revision 1
# baseline (speedup 1.0000x reference)
"""Trainium2 Bass kernel for the dual cross-attention similarity module.

Math (per query q of 300, way w of 5, fp32):
  qkv from shared W; B->A attention (l=25 b-tokens over m=125 a-tokens) and
  A->B attention (l=125 a-tokens over m=25 b-tokens); outputs are negated
  squared Frobenius distances between v and softmax-reconstructions:
    qs[q,w] = -|v_b[q] - softmax(q_b k_a^T/sqrt(d)) v_a[w]|^2
    sq[q,w] = -|v_a[w] - softmax(q_a k_b^T/sqrt(d)) v_b[q]|^2

Sharding: queries split 40/core across 8 cores (300 padded to 320 with
zeros); features_a / W / constants replicated; no collectives.

Per-core design:
  * qkv tensors are produced d-major ([128 d, tokens]) straight from the
    [C, tokens]-major feature layout, so the attention matmuls need no
    transposes. Large matmuls run with float32r operands (1 cycle/row on the
    PE when the moving free dim >= 256, vs 4 for fp32).
  * exp needs no max subtraction: |logit| <= ~1.5 for this distribution.
  * B->A: scores transposed ([m, l]); unnormalized reconstruction
    R[d,l] = sum_m v_a[m,d] E[m,l]; then per-l scalars c1 = <v_b, R>,
    c2 = |R|^2, Z = sum_m E are partition-reduced on the PE into [5, TB]
    psum accumulators (way-indicator stationaries keep outputs at base
    partition 0, since matmul outputs must start at partition 0/32/64/96).
    qs = -sum_l (n_b - 2 c1/Z + c2/Z^2).
  * A->B packs 5 queries (5x25 m-tokens) on partitions and uses the Gram
    expansion |v_a - P v_b|^2 = n_a - 2<E,G>/Z + <E, Gram_b E>/Z^2 with
    block-diagonal Gram (via a blockdiag mask) and group-shifted block-ones
    stationaries accumulating Z/U/V for all 8 groups into [40, TA] psum.
"""

import numpy as np

import concourse.bass as bass
import concourse.bacc as bacc
import concourse.tile as tile
from concourse import mybir
from concourse.bass_utils import run_bass_kernel_spmd

F32 = mybir.dt.float32
F32R = mybir.dt.float32r
AL = mybir.AluOpType
AF = mybir.ActivationFunctionType
AX = mybir.AxisListType

SCALE = 0.08838834764831845  # 1/sqrt(128)
N_CORES = 8
NQ = 40          # queries per core (8 groups of 5)
LB = 25          # b tokens per query
LA = 125         # a tokens per way
NW = 5           # n_way
TB = NQ * LB     # 1000 b tokens per core
TA = NW * LA     # 625 a tokens
TAP = NW * 128   # 640: a tokens padded to 128/way for even fp32r matmuls


def _chunks(total, step=512):
    c, out = 0, []
    while c < total:
        out.append((c, min(step, total - c)))
        c += step
    return out


def _f(ap):
    """fp32 view for odd-width matmuls (fp32r requires even moving dims)."""
    return ap.bitcast(F32)


def build_nc():
    nc = bacc.Bacc("TRN2", target_bir_lowering=False, debug=False)

    fa_d = nc.dram_tensor("fa", [640, TAP], F32R, kind="ExternalInput")
    fb_d = nc.dram_tensor("fb", [640, TB], F32R, kind="ExternalInput")
    wt_d = nc.dram_tensor("wt", [640, 384], F32R, kind="ExternalInput")
    ident_d = nc.dram_tensor("ident", [128, 128], F32R, kind="ExternalInput")
    ones128_d = nc.dram_tensor("ones128", [128, 1], F32R, kind="ExternalInput")
    ones5r_d = nc.dram_tensor("ones5r", [1, 5], F32R, kind="ExternalInput")
    ow5_d = nc.dram_tensor("ow5", [125, 5, 5], F32R, kind="ExternalInput")
    ow128_d = nc.dram_tensor("ow128", [128, 5, 5], F32R, kind="ExternalInput")
    bo40_d = nc.dram_tensor("bo40", [125, 8, 40], F32R, kind="ExternalInput")
    bdm_d = nc.dram_tensor("bdm", [125, 125], F32, kind="ExternalInput")
    half125_d = nc.dram_tensor("half125", [1, 125], F32R, kind="ExternalInput")
    sq_d = nc.dram_tensor("sq", [NQ, NW], F32, kind="ExternalOutput")
    qs_d = nc.dram_tensor("qs", [NW, NQ], F32, kind="ExternalOutput")

    with tile.TileContext(nc) as tc:
        with (
            tc.tile_pool(name="const", bufs=1) as const,
            tc.tile_pool(name="feat", bufs=1) as feat,
            tc.tile_pool(name="persist", bufs=1) as persist,
            tc.tile_pool(name="ew", bufs=1) as ew,
            tc.tile_pool(name="work", bufs=2) as work,
        ):
            # ---- loads ----
            ident = const.tile([128, 128], F32R)
            nc.sync.dma_start(out=ident, in_=ident_d[:])
            ones128 = const.tile([128, 1], F32R)
            nc.sync.dma_start(out=ones128, in_=ones128_d[:])
            ones5r = const.tile([1, 5], F32R)
            nc.sync.dma_start(out=ones5r, in_=ones5r_d[:])
            ow5 = const.tile([125, 5, 5], F32R)
            nc.sync.dma_start(out=ow5, in_=ow5_d[:])
            ow128 = const.tile([128, 5, 5], F32R)
            nc.sync.dma_start(out=ow128, in_=ow128_d[:])
            bo40 = const.tile([125, 8, 40], F32R)
            nc.sync.dma_start(out=bo40, in_=bo40_d[:])
            bdm = const.tile([125, 125], F32)
            nc.sync.dma_start(out=bdm, in_=bdm_d[:])
            half125 = const.tile([1, 125], F32R)
            nc.sync.dma_start(out=half125, in_=half125_d[:])

            wt = feat.tile([128, 5, 384], F32R)
            wt_r = wt_d.rearrange("(cb c) e -> c cb e", c=128)
            fa = feat.tile([128, 5, TAP], F32R)
            fa_r = fa_d.rearrange("(cb c) t -> c cb t", c=128)
            fb = feat.tile([128, 5, TB], F32R)
            fb_r = fb_d.rearrange("(cb c) t -> c cb t", c=128)
            for cb in range(5):
                nc.sync.dma_start(out=wt[:, cb, :], in_=wt_r[:, cb, :])
            for cb in range(5):
                nc.sync.dma_start(out=fa[:, cb, :], in_=fa_r[:, cb, :])
            for cb in range(5):
                nc.sync.dma_start(out=fb[:, cb, 0:500], in_=fb_r[:, cb, 0:500])
            for cb in range(5):
                nc.sync.dma_start(out=fb[:, cb, 500:TB], in_=fb_r[:, cb, 500:TB])

            # warm the ACT table set (exp_and_others holds exp/copy/square)
            warm = work.tile([1, 1], F32, tag="warm")
            nc.scalar.activation(out=warm, in_=ones128[0:1, 0:1], func=AF.Exp)

            # ---- phase A: qkv (d-major), token-major v_a, n_a, n_b ----
            psA_cm = tc.tile_pool(name="psA", bufs=1, space="PSUM")
            psA = psA_cm.__enter__()
            qkv_b = persist.tile([128, 3, TB], F32R)  # q/k/v for b tokens
            qkv_a = persist.tile([128, 3, TAP], F32R)
            for src, dst, total in ((fa, qkv_a, TAP), (fb, qkv_b, TB)):
                for e in range(3):
                    for c0, cn in _chunks(total, 320 if total == TAP else 500):
                        pqkv = psA.tile([128, 512], F32, tag="pqkv", bufs=3)
                        for cb in range(5):
                            nc.tensor.matmul(
                                pqkv[:, :cn],
                                (wt[:, cb, e * 128:(e + 1) * 128]),
                                (src[:, cb, c0:c0 + cn]),
                                start=(cb == 0),
                                stop=(cb == 4),
                            )
                        nc.scalar.copy(out=dst[:, e, c0:c0 + cn], in_=pqkv[:, :cn])
            qT_b, kT_b, vT_b = (qkv_b[:, i, :] for i in range(3))
            qT_a, kT_a, vT_a = (qkv_a[:, i, :] for i in range(3))

            vA_tok = persist.tile([125, 5, 128], F32R)
            for w in range(5):
                ptp = psA.tile([125, 128], F32, tag="ptp", bufs=2)
                nc.tensor.transpose(ptp, _f(vT_a[:, w * 128:w * 128 + 125]),
                                    _f(ident))
                nc.scalar.copy(out=vA_tok[:, w, :], in_=ptp)

            # n_a[l] = |v_a[w][l]|^2 ; na_rep = 0.5*n_a on 125 partitions
            sqa = work.tile([128, TAP], F32R, tag="sqa")
            nc.scalar.activation(out=sqa, in_=vT_a, func=AF.Square)
            pna = psA.tile([1, TAP], F32, tag="pwide", bufs=1,
                           padded_shape=[128, TB])
            for c0, cn in _chunks(TAP):
                nc.tensor.matmul(pna[:, c0:c0 + cn], (ones128),
                                 (sqa[:, c0:c0 + cn]), start=True, stop=True)
            na_sb = persist.tile([1, TAP], F32R)
            nc.scalar.copy(out=na_sb, in_=pna)
            na_rep = persist.tile([125, TAP], F32R)
            pnar = psA.tile([125, TAP], F32, tag="pwide", bufs=1,
                            padded_shape=[128, TB])
            for c0, cn in _chunks(TAP):
                nc.tensor.matmul(pnar[:, c0:c0 + cn], (half125),
                                 (na_sb[:, c0:c0 + cn]), start=True, stop=True)
            nc.scalar.copy(out=na_rep, in_=pnar)

            # n_b[l] = |v_b[q][l]|^2 replicated to [5, TB]
            sqb = work.tile([128, TB], F32R, tag="sqb")
            nc.scalar.activation(out=sqb, in_=vT_b, func=AF.Square)
            pnb = psA.tile([1, TB], F32, tag="pwide", bufs=1,
                           padded_shape=[128, TB])
            for c0, cn in _chunks(TB):
                nc.tensor.matmul(pnb[:, c0:c0 + cn], (ones128),
                                 (sqb[:, c0:c0 + cn]), start=True, stop=True)
            nb_sb = persist.tile([1, TB], F32R)
            nc.scalar.copy(out=nb_sb, in_=pnb)
            nbrep = persist.tile([5, TB], F32)
            pnbr = psA.tile([5, TB], F32, tag="pwide", bufs=1,
                            padded_shape=[128, TB])
            for c0, cn in _chunks(TB):
                nc.tensor.matmul(pnbr[:, c0:c0 + cn], (ones5r),
                                 (nb_sb[:, c0:c0 + cn]), start=True, stop=True)
            nc.scalar.copy(out=nbrep, in_=pnbr)

            bds = persist.tile([125, 8, 125], F32R)
            for g in range(8):
                pgram = psA.tile([125, 125], F32, tag="pgram", bufs=1)
                nc.tensor.matmul(pgram, _f(vT_b[:, g * 125:(g + 1) * 125]),
                                 _f(vT_b[:, g * 125:(g + 1) * 125]),
                                 start=True, stop=True)
                # zero the off-diagonal query blocks via a blockdiag mask
                nc.vector.tensor_mul(bds[:, g, :], pgram, bdm)

            psA_cm.__exit__(None, None, None)

            # ---- phase B: B attends A ----
            psB_cm = tc.tile_pool(name="psB", bufs=1, space="PSUM")
            psB = psB_cm.__enter__()
            qs_sb = work.tile([5, NQ], F32, tag="qs_sb")
            for ci, (c0, cn) in enumerate(_chunks(TB, 500)):
                ZC = psB.tile([5, 500], F32, tag="ZC", bufs=1)
                C1 = psB.tile([5, 500], F32, tag="C1", bufs=1)
                C2 = psB.tile([5, 500], F32, tag="C2", bufs=1)
                for w in range(5):
                    sba = psB.tile([125, 500], F32, tag="sba", bufs=2)
                    nc.tensor.matmul(sba[:, :cn],
                                     kT_a[:, w * 128:w * 128 + 125],
                                     qT_b[:, c0:c0 + cn],
                                     start=True, stop=True)
                    e_w = work.tile([125, 500], F32R, tag="e_w", bufs=3)
                    nc.scalar.activation(out=e_w[:, :cn], in_=sba[:, :cn],
                                         func=AF.Exp, scale=SCALE)
                    nc.tensor.matmul(ZC[:, :cn], (ow5[:, w, :]),
                                     (e_w[0:125, :cn]),
                                     start=(w == 0), stop=(w == 4))
                    rp = psB.tile([128, 500], F32, tag="rp", bufs=3)
                    nc.tensor.matmul(rp[:, :cn], (vA_tok[:, w, :]),
                                     (e_w[:, :cn]), start=True, stop=True)
                    c1sb = work.tile([128, 500], F32R, tag="c1sb", bufs=3)
                    nc.vector.tensor_mul(c1sb[:, :cn], rp[:, :cn],
                                         vT_b[:, c0:c0 + cn])
                    nc.tensor.matmul(C1[:, :cn], (ow128[:, w, :]),
                                     (c1sb[:, :cn]),
                                     start=(w == 0), stop=(w == 4))
                    c2sb = work.tile([128, 500], F32R, tag="c2sb", bufs=3)
                    nc.scalar.activation(out=c2sb[:, :cn], in_=rp[:, :cn],
                                         func=AF.Square)
                    nc.tensor.matmul(C2[:, :cn], (ow128[:, w, :]),
                                     (c2sb[:, :cn]),
                                     start=(w == 0), stop=(w == 4))

                # qs = -sum_{l in q} (n_b - 2 c1 r + c2 r^2),  r = 1/Z
                rba = work.tile([5, 500], F32, tag="rba")
                nc.vector.reciprocal(out=rba[:, :cn], in_=ZC[0:5, :cn])
                t1 = work.tile([5, 500], F32, tag="t1")
                nc.vector.tensor_mul(t1[:, :cn], C2[0:5, :cn], rba[:, :cn])
                t2 = work.tile([5, 500], F32, tag="t2")
                nc.vector.scalar_tensor_tensor(out=t2[:, :cn], in0=C1[0:5, :cn],
                                               scalar=-2.0, in1=t1[:, :cn],
                                               op0=AL.mult, op1=AL.add)
                t3 = work.tile([5, 500], F32, tag="t3")
                nc.vector.tensor_mul(t3[:, :cn], t2[:, :cn], rba[:, :cn])
                fba = work.tile([5, 500], F32, tag="fba")
                nc.vector.tensor_add(fba[:, :cn], t3[:, :cn],
                                     nbrep[:, c0:c0 + cn])
                nq_c = cn // LB
                nc.vector.tensor_reduce(
                    out=qs_sb[:, ci * 20:ci * 20 + nq_c],
                    in_=fba[:, :cn].rearrange("p (q l) -> p q l", q=nq_c),
                    op=AL.add, axis=AX.X, negate=True)
            nc.sync.dma_start(out=qs_d[:], in_=qs_sb)
            psB_cm.__exit__(None, None, None)

            # ---- phase C: A attends B (query-packed Gram expansion) ----
            psC_cm = tc.tile_pool(name="psC", bufs=1, space="PSUM")
            psC = psC_cm.__enter__()

            # l chunks of 320 (2.5 ways); way w spans cols [128w, 128w+125).
            sq_sb = work.tile([NQ, NW], F32, tag="sq_sb")
            w2part = work.tile([NQ, 2], F32, tag="w2part")
            for ci, (c0, cn) in enumerate(_chunks(TAP, 320)):
                Zp = psC.tile([NQ, 320], F32, tag="Zp", bufs=1)
                Up = psC.tile([NQ, 320], F32, tag="Up", bufs=1)
                Vp = psC.tile([NQ, 320], F32, tag="Vp", bufs=1)
                for g in range(8):
                    gsl = slice(g * 125, (g + 1) * 125)
                    sab = psC.tile([125, 320], F32, tag="pab", bufs=4)
                    nc.tensor.matmul(sab[:, :cn], kT_b[:, gsl],
                                     qT_a[:, c0:c0 + cn], start=True, stop=True)
                    eg = work.tile([125, 320], F32R, tag="eg", bufs=3)
                    nc.scalar.activation(out=eg[:, :cn], in_=sab[:, :cn],
                                         func=AF.Exp, scale=SCALE)

                    gab = psC.tile([125, 320], F32, tag="pab", bufs=4)
                    nc.tensor.matmul(gab[:, :cn], vT_b[:, gsl],
                                     vT_a[:, c0:c0 + cn], start=True, stop=True)
                    gpp = work.tile([125, 320], F32R, tag="gpp", bufs=3)
                    nc.vector.tensor_sub(gpp[:, :cn], gab[:, :cn],
                                         na_rep[:, c0:c0 + cn])

                    aeb = psC.tile([125, 320], F32, tag="pab", bufs=4)
                    nc.tensor.matmul(aeb[:, :cn], bds[:, g, :],
                                     eg[:, :cn], start=True, stop=True)

                    egp = work.tile([125, 320], F32R, tag="egp", bufs=3)
                    nc.gpsimd.tensor_mul(egp[:, :cn], eg[:, :cn], gpp[:, :cn])
                    eab = work.tile([125, 320], F32R, tag="eab", bufs=3)
                    nc.vector.tensor_mul(eab[:, :cn], eg[:, :cn], aeb[:, :cn])

                    for rhs, dst in ((eg, Zp), (egp, Up), (eab, Vp)):
                        nc.tensor.matmul(dst[:, :cn], bo40[:, g, :],
                                         rhs[:, :cn],
                                         start=(g == 0), stop=(g == 7))

                # sq = sum_l (2*U*r - V*r^2) with r = 1/Z; way 2 straddles the
                # chunk boundary (cols 256..381 global), summed via w2part.
                rab = work.tile([NQ, 320], F32, tag="rab")
                nc.vector.reciprocal(out=rab[:, :cn], in_=Zp[0:NQ, :cn])
                u1 = work.tile([NQ, 320], F32, tag="u1")
                nc.vector.tensor_mul(u1[:, :cn], Vp[0:NQ, :cn], rab[:, :cn])
                u2 = work.tile([NQ, 320], F32, tag="u2")
                nc.vector.scalar_tensor_tensor(out=u2[:, :cn], in0=Up[0:NQ, :cn],
                                               scalar=2.0, in1=u1[:, :cn],
                                               op0=AL.mult, op1=AL.subtract)
                f2 = work.tile([NQ, 320], F32, tag="f2")
                nc.vector.tensor_mul(f2[:, :cn], u2[:, :cn], rab[:, :cn])
                if ci == 0:
                    # ways 0,1 full; way 2 cols 256..320 partial
                    nc.vector.tensor_reduce(
                        out=sq_sb[:, 0:2],
                        in_=f2[:, 0:256].rearrange("p (w l) -> p w l", w=2)[:, :, 0:125],
                        op=AL.add, axis=AX.X)
                    nc.vector.tensor_reduce(
                        out=w2part[:, 0:1],
                        in_=f2[:, 256:320].rearrange("p (o l) -> p o l", o=1),
                        op=AL.add, axis=AX.X)
                else:
                    # way 2 cols 0..61 local (global 320..381); ways 3,4 full
                    nc.vector.tensor_reduce(
                        out=w2part[:, 1:2],
                        in_=f2[:, 0:61].rearrange("p (o l) -> p o l", o=1),
                        op=AL.add, axis=AX.X)
                    nc.vector.tensor_reduce(
                        out=sq_sb[:, 3:5],
                        in_=f2[:, 64:320].rearrange("p (w l) -> p w l", w=2)[:, :, 0:125],
                        op=AL.add, axis=AX.X)
            nc.vector.tensor_add(sq_sb[:, 2:3], w2part[:, 0:1], w2part[:, 1:2])
            nc.sync.dma_start(out=sq_d[:], in_=sq_sb)
            psC_cm.__exit__(None, None, None)

    nc.compile()
    return nc


_CACHE = {}
_last_in_maps = None


def _get_nc():
    if "nc" not in _CACHE:
        _CACHE["nc"] = build_nc()
    return _CACHE["nc"]


def _consts():
    ident = np.eye(128, dtype=np.float32)
    ones128 = np.ones((128, 1), np.float32)
    ones5r = np.ones((1, 5), np.float32)
    ow5 = np.zeros((125, 5, 5), np.float32)
    ow128 = np.zeros((128, 5, 5), np.float32)
    for w in range(5):
        ow5[:, w, w] = 1.0
        ow128[:, w, w] = 1.0
    bo125 = np.kron(np.eye(5, dtype=np.float32), np.ones((25, 1), np.float32))
    bo40 = np.zeros((125, 8, 40), np.float32)
    for g in range(8):
        bo40[:, g, 5 * g:5 * g + 5] = bo125
    bdm = np.kron(np.eye(5, dtype=np.float32), np.ones((25, 25), np.float32))
    half125 = np.full((1, 125), 0.5, np.float32)
    return dict(ident=ident, ones128=ones128, ones5r=ones5r, ow5=ow5,
                ow128=ow128, bo40=bo40, bdm=bdm, half125=half125)


def kernel(features_a, features_b, W):
    global _last_in_maps
    features_a = np.asarray(features_a, np.float32)
    features_b = np.asarray(features_b, np.float32)
    W = np.asarray(W, np.float32)

    nq_total = features_b.shape[0]
    fbp = np.zeros((N_CORES * NQ, 640, LB), np.float32)
    fbp[:nq_total] = features_b
    fb_t = np.ascontiguousarray(fbp.transpose(1, 0, 2))  # [640, 320, 25]
    fa_pad = np.zeros((640, NW, 128), np.float32)
    fa_pad[:, :, :LA] = features_a.transpose(1, 0, 2)
    fa_t = np.ascontiguousarray(fa_pad.reshape(640, TAP))
    wt = np.ascontiguousarray(W.T)
    consts = _consts()

    in_maps = []
    for c in range(N_CORES):
        m = {
            "fa": fa_t,
            "fb": np.ascontiguousarray(
                fb_t[:, c * NQ:(c + 1) * NQ, :]).reshape(640, TB),
            "wt": wt,
        }
        m.update(consts)
        in_maps.append(m)

    _last_in_maps = in_maps
    nc = _get_nc()
    res = run_bass_kernel_spmd(nc, in_maps, core_ids=list(range(N_CORES)))

    sq = np.zeros((N_CORES * NQ, NW), np.float32)
    qs = np.zeros((N_CORES * NQ, NW), np.float32)
    for c in range(N_CORES):
        sq[c * NQ:(c + 1) * NQ] = res.results[c]["sq"]
        qs[c * NQ:(c + 1) * NQ] = res.results[c]["qs"].T
    return sq[:nq_total], qs[:nq_total]



# revision 31
# speedup vs baseline: 1.1602x; 1.1602x over previous
"""Trainium2 Bass kernel for the dual cross-attention similarity module.

Math (per query q of 300, way w of 5):
  qkv from shared W; B->A attention (25 b-tokens over 125 a-tokens/way) and
  A->B attention (125 a-tokens/way over 25 b-tokens); outputs are negated
  squared Frobenius distances between v and softmax-reconstructions.

Sharding: queries split 40/core across 8 cores (300 padded to 320);
features_a / W / consts replicated; no collectives.

Per-core design (v2):
  * Inputs quantized host-side to fp8e4m3 (W pre-scaled x2; outputs
    unscaled /4 on device via the final ops' scale) -> 1.3MB load.
  * qkv runs as fp8 DoubleRow matmuls over a 768-padded contraction
    (3 pair-passes instead of 5 fp32r passes).
  * q/k (and v for gram use) are retiled to [64, 2, T] fp8 via sbuf->sbuf
    DMAs so the score matmuls (s_ba, s_ab) and gram matmuls (v_b^T v_a,
    blockdiag v_b^T v_b) also run DoubleRow at 0.5 cyc/row.
  * n_a / n_b terms are independent of the softmax normalizer, so they fold
    into per-way / per-query constants instead of broadcast tensors.
  * B->A: per 500-col chunk, 5 ways of scores in two psum tiles, one fused
    exp per tile; R = V_a E, then Z/c1/c2 partition-reduced into one psum
    bank (rows 0-4/32-36/64-68). Tail: Z/C1/C2 + n_b stacked [16,1000] in
    sbuf, PE-transposed to [128,8,16], rational math on 40-col tiles, and
    indicator matmuls reduce 25-token groups -> [40, 6].
  * A->B: way-aligned a-chunks (384, 256); per group g: scores + cross-gram
    DR matmuls, aeb = blockdiag-gram @ E, Z/U/V accumulated via bo40; tail
    reads psum directly, reduces [40, nw, 128 -> 0:125] slices.
"""

import numpy as np
import ml_dtypes

import concourse.bass as bass
import concourse.bacc as bacc
import concourse.tile as tile
from concourse import mybir
from concourse.bass_utils import run_bass_kernel_spmd

F32 = mybir.dt.float32
BF16 = mybir.dt.bfloat16
FP8 = mybir.dt.float8e4
AL = mybir.AluOpType
AF = mybir.ActivationFunctionType
AX = mybir.AxisListType
DR = mybir.MatmulPerfMode.DoubleRow

WS = 2.0                      # host-side W scale (outputs carry WS^2)
SCALE = 0.08838834764831845 / (WS * WS)   # 1/sqrt(128) / WS^2
N_CORES = 8
NQ = 40          # queries per core
LB = 25          # b tokens per query
LA = 125         # a tokens per way
NW = 5
TB = NQ * LB     # 1000
TAP = NW * 128   # 640 (a tokens padded to 128/way)

# consts blob (bf16) column offsets
C_ID = 0          # ident [128,128]
C_OW5 = 128       # [125,5,5]
C_OW128 = 153     # [128,5,5]
C_BO40 = 178      # [125,8,40]
C_BDM = 498       # [125,125]
C_ONES = 623      # [128,1]
C_ONES40 = 624    # [1,40]
NCB = 664
# f32 blob offsets
F_IND = 0         # [128,8,40]
F_ID = 320        # [128,128]
NCF = 448


def build_nc():
    nc = bacc.Bacc("TRN2", target_bir_lowering=False, debug=False)

    fa_d = nc.dram_tensor("fa", [640, TAP], FP8, kind="ExternalInput")
    fb_d = nc.dram_tensor("fb", [640, TB], FP8, kind="ExternalInput")
    wt_d = nc.dram_tensor("wt", [640, 384], FP8, kind="ExternalInput")
    cb_d = nc.dram_tensor("cb", [128, NCB], BF16, kind="ExternalInput")
    cf_d = nc.dram_tensor("cf", [128, NCF], F32, kind="ExternalInput")
    sq_d = nc.dram_tensor("sq", [NQ, NW], F32, kind="ExternalOutput")
    qs_d = nc.dram_tensor("qs", [NQ, NW], F32, kind="ExternalOutput")

    with tile.TileContext(nc) as tc:
        with (
            tc.tile_pool(name="const", bufs=1) as const,
            tc.tile_pool(name="feat", bufs=1) as feat,
            tc.tile_pool(name="persist", bufs=1) as persist,
            tc.tile_pool(name="ew", bufs=1) as ew,
            tc.tile_pool(name="work", bufs=2) as work,
        ):
            # ---------------- loads ----------------
            cb = const.tile([128, NCB], BF16)
            nc.sync.dma_start(out=cb, in_=cb_d[:])
            cf = const.tile([128, NCF], F32)
            nc.sync.dma_start(out=cf, in_=cf_d[:])
            ident = cb[:, C_ID:C_ID + 128]
            ow5 = cb[0:125, C_OW5:C_OW5 + 25].rearrange("p (w c) -> p w c", w=5)
            ow128 = cb[:, C_OW128:C_OW128 + 25].rearrange("p (w c) -> p w c", w=5)
            bo40 = cb[0:125, C_BO40:C_BO40 + 320].rearrange("p (g c) -> p g c", g=8)
            bdm = cb[0:125, C_BDM:C_BDM + 125]
            ones128 = cb[:, C_ONES:C_ONES + 1]
            ones40 = cb[0:1, C_ONES40:C_ONES40 + 40]
            indB = cf[:, F_IND:F_IND + 320].rearrange("p (g c) -> p g c", g=8)
            identf = cf[:, F_ID:F_ID + 128]

            wt = feat.tile([128, 6, 384], FP8)
            nc.gpsimd.memset(wt[:, 5, :], 0.0)
            nc.sync.dma_start(out=wt[:, 0:5, :],
                              in_=wt_d.rearrange("(cb c) e -> c cb e", c=128))
            fa = feat.tile([128, 6, TAP], FP8)
            nc.gpsimd.memset(fa[:, 5, :], 0.0)
            nc.sync.dma_start(out=fa[:, 0:5, :],
                              in_=fa_d.rearrange("(cb c) t -> c cb t", c=128))
            fb = feat.tile([128, 6, TB], FP8)
            nc.gpsimd.memset(fb[:, 5, :], 0.0)
            nc.sync.dma_start(out=fb[:, 0:5, :],
                              in_=fb_d.rearrange("(cb c) t -> c cb t", c=128))

            # warm ACT tables (exp/copy/square live in one set)
            warm = work.tile([1, 1], F32, tag="warm")
            nc.scalar.activation(out=warm, in_=cb[0:1, 0:1], func=AF.Exp)

            # ---------------- phase A: qkv ----------------
            # DoubleRow over 768-padded contraction: pairs (0,1),(2,3),(4,5)
            qa8 = persist.tile([128, TAP], FP8)
            ka8 = persist.tile([128, TAP], FP8)
            vaB = persist.tile([128, TAP], BF16)
            va8 = persist.tile([128, TAP], FP8)
            qb8 = persist.tile([128, TB], FP8)
            kb8 = persist.tile([128, TB], FP8)
            vbB = persist.tile([128, TB], BF16)
            vb8 = persist.tile([128, TB], FP8)

            psA_cm = tc.tile_pool(name="psA", bufs=1, space="PSUM")
            psA = psA_cm.__enter__()

            def qkv_emit(src, total, chunks, dests):
                # dests[e] = list of (tile, dtype-ignored) targets for e-th 128-slice
                for e in range(3):
                    done = 0
                    for grp in chunks:   # grp: list of chunk widths sharing a psum tile
                        gw = sum(grp)
                        pq = psA.tile([128, 512], F32, tag="pqkv", bufs=3)
                        off = 0
                        for cw in grp:
                            c0 = done + off
                            for j in range(3):
                                nc.tensor.matmul(
                                    pq[:, off:off + cw],
                                    wt[:, 2 * j:2 * j + 2, e * 128:(e + 1) * 128],
                                    src[:, 2 * j:2 * j + 2, c0:c0 + cw],
                                    start=(j == 0), stop=(j == 2),
                                    perf_mode=DR,
                                )
                            off += cw
                        for k, dst in enumerate(dests[e]):
                            if (e + k) % 2 == 0:
                                nc.scalar.copy(out=dst[:, done:done + gw],
                                               in_=pq[:, 0:gw])
                            else:
                                nc.vector.tensor_copy(out=dst[:, done:done + gw],
                                                      in_=pq[:, 0:gw])
                        done += gw

            qkv_emit(fa, TAP, [[256, 256], [128]],
                     [[qa8], [ka8], [vaB, va8]])
            qkv_emit(fb, TB, [[250, 250], [250, 250]],
                     [[qb8], [kb8], [vbB, vb8]])

            # retiles to DoubleRow layout [64, 2, T]
            kaD = persist.tile([64, 2, TAP], FP8)
            qaD = persist.tile([64, 2, TAP], FP8)
            vaD = persist.tile([64, 2, TAP], FP8)
            qbD = persist.tile([64, 2, TB], FP8)
            for src, dst in ((ka8, kaD), (qa8, qaD), (va8, vaD),
                             (qb8, qbD)):
                nc.sync.dma_start(out=dst[:, 0, :], in_=src[0:64, :])
                nc.sync.dma_start(out=dst[:, 1, :], in_=src[64:128, :])
            # k_b / v_b retiled with 128-padded 125-token groups (for even
            # DoubleRow stationaries in phase C)
            kbD = persist.tile([64, 2, 8, 128], FP8)
            vbD = persist.tile([64, 2, 8, 128], FP8)
            for src, dst in ((kb8, kbD), (vb8, vbD)):
                nc.gpsimd.memset(dst[:, :, :, 125:128], 0.0)
                nc.sync.dma_start(
                    out=dst[:, 0, :, 0:125],
                    in_=src[0:64, :].rearrange("p (g t) -> p g t", g=8))
                nc.sync.dma_start(
                    out=dst[:, 1, :, 0:125],
                    in_=src[64:128, :].rearrange("p (g t) -> p g t", g=8))

            # vA_tok[m, w, d] via transposes (bf16 psum)
            ptr_cm = tc.tile_pool(name="ptr", bufs=1, space="PSUM")
            ptr = ptr_cm.__enter__()
            vA_tok = persist.tile([125, 5, 128], BF16)
            ptp = ptr.tile([125, 5, 128], BF16, tag="ptp")
            for w in range(5):
                nc.tensor.transpose(ptp[:, w, :],
                                    vaB[:, w * 128:w * 128 + 125], ident)
            nc.vector.tensor_copy(out=vA_tok, in_=ptp)

            # n_a (per way) and n_b (per b-token)
            sqa = work.tile([128, TAP], BF16, tag="sqa")
            nc.gpsimd.tensor_mul(sqa, vaB, vaB)
            pna = ptr.tile([1, TAP], F32, tag="pna", padded_shape=[128, TAP])
            nc.tensor.matmul(pna[:, 0:512], ones128, sqa[:, 0:512],
                             start=True, stop=True)
            nc.tensor.matmul(pna[:, 512:640], ones128, sqa[:, 512:640],
                             start=True, stop=True)
            naF = work.tile([1, 5], F32, tag="naF")
            nc.vector.tensor_reduce(
                out=naF, in_=pna.rearrange("p (w l) -> p w l", w=5),
                op=AL.add, axis=AX.X)
            naB = persist.tile([1, 5], BF16)
            nc.scalar.copy(out=naB, in_=naF)

            sqb = work.tile([128, TB], BF16, tag="sqb")
            nc.gpsimd.tensor_mul(sqb, vbB, vbB)
            pnb = ptr.tile([1, 2, 512], F32, tag="pnb", padded_shape=[128, 2, 512])
            nc.tensor.matmul(pnb[:, 0, 0:500], ones128, sqb[:, 0:500],
                             start=True, stop=True)
            nc.tensor.matmul(pnb[:, 1, 0:500], ones128, sqb[:, 500:1000],
                             start=True, stop=True)
            # n_b row evacuated to sbuf for the tail transposes
            nbS = persist.tile([1, 1024], F32)
            nc.gpsimd.memset(nbS[:, TB:1024], 0.0)
            nc.scalar.copy(out=nbS[:, 0:TB].rearrange("p (two c) -> p two c", two=2),
                           in_=pnb[:, :, 0:500])
            ptr_cm.__exit__(None, None, None)

            # blockdiag gram of v_b per group
            ptr2_cm = tc.tile_pool(name="ptr2", bufs=1, space="PSUM")
            ptr2 = ptr2_cm.__enter__()
            bds = persist.tile([125, 8, 125], BF16)
            for g in range(8):
                gsl = slice(g * 125, (g + 1) * 125)
                pgram = ptr2.tile([125, 128], F32, tag="pgram", bufs=2)
                nc.tensor.matmul(pgram[:, 0:125], vb8[:, gsl], vb8[:, gsl],
                                 start=True, stop=True)
                nc.vector.tensor_mul(bds[:, g, :], pgram[:, 0:125], bdm)
            ptr2_cm.__exit__(None, None, None)
            psA_cm.__exit__(None, None, None)

            # ---------------- phase B: B attends A ----------------
            psB_cm = tc.tile_pool(name="psB", bufs=1, space="PSUM")
            psB = psB_cm.__enter__()
            zs = persist.tile([69, 1024], F32)  # rows 0:5 Z, 32:37 C1, 64:69 C2
            nc.gpsimd.memset(zs[:, TB:1024], 1.0)
            # psum: sbaA 2 banks, sbaB 3, rp 2, zcc 1  -> 8
            sbaA = psB.tile([128, 2, 512], F32, tag="sbaA", bufs=1)
            sbaB = psB.tile([128, 3, 512], F32, tag="sbaB", bufs=1)
            zcc = psB.tile([128, 512], F32, tag="zcc", bufs=1)
            for ci in range(2):
                c0 = ci * 500
                # scores for all 5 ways (DoubleRow, 250-col halves)
                for w in range(5):
                    dst = sbaA[:, w, :] if w < 2 else sbaB[:, w - 2, :]
                    for h in range(2):
                        nc.tensor.matmul(
                            dst[:, h * 250:(h + 1) * 250],
                            kaD[:, :, w * 128:(w + 1) * 128],
                            qbD[:, :, c0 + h * 250:c0 + (h + 1) * 250],
                            start=True, stop=True, perf_mode=DR)
                eA = ew.tile([128, 2, 500], BF16, tag="eA", bufs=1)
                nc.scalar.activation(out=eA, in_=sbaA[:, :, 0:500],
                                     func=AF.Exp, scale=SCALE)
                eB = ew.tile([128, 3, 500], BF16, tag="eB", bufs=1)
                nc.scalar.activation(out=eB, in_=sbaB[:, :, 0:500],
                                     func=AF.Exp, scale=SCALE)
                for w in range(5):
                    e_w = (eA[:, w, :] if w < 2 else eB[:, w - 2, :])[0:125, :]
                    rp = psB.tile([128, 512], F32, tag="rp", bufs=2)
                    nc.tensor.matmul(rp[:, 0:500], vA_tok[:, w, :], e_w,
                                     start=True, stop=True)
                    c1sb = work.tile([128, 500], BF16, tag="c1sb", bufs=2)
                    nc.vector.tensor_mul(c1sb, rp[:, 0:500],
                                         vbB[:, c0:c0 + 500])
                    c2sb = work.tile([128, 500], BF16, tag="c2sb", bufs=2)
                    nc.scalar.activation(out=c2sb, in_=rp[:, 0:500],
                                         func=AF.Square)
                    nc.tensor.matmul(zcc[0:5, 0:500], ow5[:, w, :], e_w,
                                     start=(w == 0), stop=(w == 4))
                    nc.tensor.matmul(zcc[32:37, 0:500], ow128[:, w, :], c1sb,
                                     start=(w == 0), stop=(w == 4))
                    nc.tensor.matmul(zcc[64:69, 0:500], ow128[:, w, :], c2sb,
                                     start=(w == 0), stop=(w == 4))
                # evacuate Z/C1/C2 accumulator rows (partition-preserving)
                nc.scalar.copy(out=zs[:, c0:c0 + 500], in_=zcc[0:69, 0:500])
            psB_cm.__exit__(None, None, None)

            # ---------------- phase C: A attends B ----------------
            psC_cm = tc.tile_pool(name="psC", bufs=1, space="PSUM")
            psC = psC_cm.__enter__()
            # psum: sab pairs 2x2 banks, gab/aeb pool 2, zuv 2 -> 8
            sq_parts = work.tile([40, 5], F32, tag="sq_parts")
            napb_ap = None
            for ci, (c0, cn) in enumerate(((0, 384), (384, 256))):
                nwc = cn // 128
                zuv = psC.tile([128, 512], F32, tag="zuv", bufs=1)
                vp = psC.tile([128, 512], F32, tag="vp", bufs=1)
                # zuv rows: 0:40 Z, 64:104 U ; vp rows: 0:40 V, 64:104 na
                if ci == 1:
                    napb_ap = vp[64:104, 0:5]
                    nc.tensor.matmul(napb_ap, ones40, naB,
                                     start=True, stop=True)
                for gp in range(4):
                    sab = psC.tile([128, 2, 512], F32, tag="sab", bufs=2)
                    for gg in range(2):
                        g = gp * 2 + gg
                        for h in range(2):
                            h0, hn = h * (cn // 2), cn // 2
                            nc.tensor.matmul(
                                sab[:, gg, h0:h0 + hn],
                                kbD[:, :, g, :],
                                qaD[:, :, c0 + h0:c0 + h0 + hn],
                                start=True, stop=True, perf_mode=DR)
                    eg2 = ew.tile([128, 2, 384], BF16, tag="eg2", bufs=2)
                    nc.scalar.activation(out=eg2[:, :, 0:cn],
                                         in_=sab[:, :, 0:cn],
                                         func=AF.Exp, scale=SCALE)
                    for gg in range(2):
                        g = gp * 2 + gg
                        e_g = eg2[0:125, gg, 0:cn]
                        gab = psC.tile([128, 512], F32, tag="gab", bufs=2)
                        for h in range(2):
                            h0, hn = h * (cn // 2), cn // 2
                            nc.tensor.matmul(
                                gab[:, h0:h0 + hn], vbD[:, :, g, :],
                                vaD[:, :, c0 + h0:c0 + h0 + hn],
                                start=True, stop=True, perf_mode=DR)
                        egp = work.tile([125, 384], BF16, tag="egp", bufs=2)
                        nc.vector.tensor_mul(egp[:, 0:cn], e_g,
                                             gab[0:125, 0:cn])
                        aeb = psC.tile([128, 512], F32, tag="gab", bufs=2)
                        nc.tensor.matmul(aeb[0:125, 0:cn], bds[:, g, :], e_g,
                                         start=True, stop=True)
                        eab = work.tile([125, 384], BF16, tag="eab", bufs=2)
                        nc.vector.tensor_mul(eab[:, 0:cn], e_g,
                                             aeb[0:125, 0:cn])
                        nc.tensor.matmul(zuv[0:40, 0:cn], bo40[:, g, :], e_g,
                                         start=(g == 0), stop=(g == 7))
                        nc.tensor.matmul(zuv[64:104, 0:cn], bo40[:, g, :],
                                         egp[:, 0:cn],
                                         start=(g == 0), stop=(g == 7))
                        nc.tensor.matmul(vp[0:40, 0:cn], bo40[:, g, :],
                                         eab[:, 0:cn],
                                         start=(g == 0), stop=(g == 7))
                # tail: f = (2U - V r) r summed over each way's 125 cols
                rab = work.tile([40, 384], F32, tag="rab")
                nc.vector.reciprocal(out=rab[:, 0:cn], in_=zuv[0:40, 0:cn])
                u1 = work.tile([40, 384], F32, tag="u1")
                nc.vector.tensor_mul(u1[:, 0:cn], vp[0:40, 0:cn],
                                     rab[:, 0:cn])
                u2 = work.tile([40, 384], F32, tag="u2")
                nc.vector.scalar_tensor_tensor(
                    out=u2[:, 0:cn], in0=zuv[64:104, 0:cn], scalar=2.0,
                    in1=u1[:, 0:cn], op0=AL.mult, op1=AL.subtract)
                f2 = work.tile([40, 384], F32, tag="f2")
                nc.vector.tensor_mul(f2[:, 0:cn], u2[:, 0:cn], rab[:, 0:cn])
                nc.vector.tensor_reduce(
                    out=sq_parts[:, ci * 3:ci * 3 + nwc],
                    in_=f2[:, 0:cn].rearrange(
                        "p (w l) -> p w l", w=nwc)[:, :, 0:125],
                    op=AL.add, axis=AX.X)
            sqt = work.tile([40, 5], F32, tag="sqt")
            nc.vector.tensor_sub(sqt, sq_parts, napb_ap)
            sq_sb = work.tile([40, 5], F32, tag="sq_sb")
            nc.scalar.activation(out=sq_sb, in_=sqt, func=AF.Copy,
                                 scale=1.0 / (WS * WS))
            nc.sync.dma_start(out=sq_d[:], in_=sq_sb)
            psC_cm.__exit__(None, None, None)

            # ---- B tail: transpose Z/C1/C2/nb, rational math, indicator ----
            psT_cm = tc.tile_pool(name="psT", bufs=1, space="PSUM")
            psT = psT_cm.__enter__()
            TtA = psT.tile([128, 4, 128], F32, tag="TtA")
            TtB = psT.tile([128, 4, 128], F32, tag="TtB")
            for s in range(8):
                Tt = TtA if s < 4 else TtB
                nc.tensor.transpose(Tt[:, s % 4, 0:69],
                                    zs[:, s * 128:(s + 1) * 128],
                                    identf[0:69, 0:69])
                nc.tensor.transpose(Tt[:, s % 4, 69:70],
                                    nbS[:, s * 128:(s + 1) * 128],
                                    identf[0:1, 0:1])
            TtS = persist.tile([128, 8, 70], F32)
            nc.scalar.copy(out=TtS[:, 0:4, :], in_=TtA[:, :, 0:70])
            nc.scalar.copy(out=TtS[:, 4:8, :], in_=TtB[:, :, 0:70])
            rT = work.tile([128, 8, 5], F32, tag="rT")
            nc.vector.reciprocal(out=rT, in_=TtS[:, :, 0:5])
            u1T = work.tile([128, 8, 5], F32, tag="u1T")
            nc.vector.tensor_mul(u1T, TtS[:, :, 64:69], rT)
            t3 = work.tile([128, 8, 5], F32, tag="t3")
            nc.vector.scalar_tensor_tensor(out=t3, in0=TtS[:, :, 32:37],
                                           scalar=-2.0, in1=u1T,
                                           op0=AL.mult, op1=AL.add)
            nc.vector.tensor_mul(TtS[:, :, 64:69], t3, rT)
            qsp = psT.tile([40, 6], F32, tag="qsp")
            for s in range(8):
                nc.tensor.matmul(qsp, indB[:, s, :], TtS[:, s, 64:70],
                                 start=(s == 0), stop=(s == 7))
            qsb = work.tile([40, 6], F32, tag="qsb")
            nc.vector.tensor_copy(out=qsb, in_=qsp)
            qs_sb = work.tile([40, 5], F32, tag="qs_sb")
            nc.vector.tensor_scalar(
                out=qs_sb, in0=qsb[:, 0:5], scalar1=qsb[:, 5:6],
                scalar2=-1.0 / (WS * WS), op0=AL.add, op1=AL.mult)
            nc.sync.dma_start(out=qs_d[:], in_=qs_sb)
            psT_cm.__exit__(None, None, None)


    nc.compile()
    return nc


_CACHE = {}


def _get_nc():
    if "nc" not in _CACHE:
        _CACHE["nc"] = build_nc()
    return _CACHE["nc"]


def _consts():
    cb = np.zeros((128, NCB), np.float32)
    cb[:, C_ID:C_ID + 128] = np.eye(128)
    ow5 = np.zeros((125, 5, 5), np.float32)
    ow128 = np.zeros((128, 5, 5), np.float32)
    for w in range(5):
        ow5[:, w, w] = 1.0
        ow128[:, w, w] = 1.0
    cb[0:125, C_OW5:C_OW5 + 25] = ow5.reshape(125, 25)
    cb[:, C_OW128:C_OW128 + 25] = ow128.reshape(128, 25)
    bo125 = np.kron(np.eye(5, dtype=np.float32), np.ones((25, 1), np.float32))
    bo40 = np.zeros((125, 8, 40), np.float32)
    for g in range(8):
        bo40[:, g, 5 * g:5 * g + 5] = bo125
    cb[0:125, C_BO40:C_BO40 + 320] = bo40.reshape(125, 320)
    cb[0:125, C_BDM:C_BDM + 125] = np.kron(
        np.eye(5, dtype=np.float32), np.ones((25, 25), np.float32))
    cb[:, C_ONES] = 1.0
    cb[0, C_ONES40:C_ONES40 + 40] = 1.0

    cf = np.zeros((128, NCF), np.float32)
    indB = np.zeros((128, 8, 40), np.float32)
    for s in range(8):
        for p in range(128):
            l = s * 128 + p
            if l < TB:
                indB[p, s, l // 25] = 1.0
    cf[:, F_IND:F_IND + 320] = indB.reshape(128, 320)
    cf[:, F_ID:F_ID + 128] = np.eye(128)
    return (cb.astype(ml_dtypes.bfloat16), cf)


def kernel(features_a, features_b, W):
    features_a = np.asarray(features_a, np.float32)
    features_b = np.asarray(features_b, np.float32)
    W = np.asarray(W, np.float32)
    f8 = ml_dtypes.float8_e4m3

    nq_total = features_b.shape[0]
    fbp = np.zeros((N_CORES * NQ, 640, LB), np.float32)
    fbp[:nq_total] = features_b
    fb_t = np.ascontiguousarray(fbp.transpose(1, 0, 2)).astype(f8)
    fa_pad = np.zeros((640, NW, 128), np.float32)
    fa_pad[:, :, :LA] = features_a.transpose(1, 0, 2)
    fa_t = np.ascontiguousarray(fa_pad.reshape(640, TAP)).astype(f8)
    wt = np.ascontiguousarray(W.T * WS).astype(f8)
    cb, cf = _consts()

    in_maps = []
    for c in range(N_CORES):
        m = {
            "fa": fa_t,
            "fb": np.ascontiguousarray(
                fb_t[:, c * NQ:(c + 1) * NQ, :]).reshape(640, TB),
            "wt": wt,
            "cb": cb,
            "cf": cf,
        }
        in_maps.append(m)

    nc = _get_nc()
    res = run_bass_kernel_spmd(nc, in_maps, core_ids=list(range(N_CORES)))

    sq = np.zeros((N_CORES * NQ, NW), np.float32)
    qs = np.zeros((N_CORES * NQ, NW), np.float32)
    for c in range(N_CORES):
        sq[c * NQ:(c + 1) * NQ] = res.results[c]["sq"]
        qs[c * NQ:(c + 1) * NQ] = res.results[c]["qs"]
    return sq[:nq_total], qs[:nq_total]


# revision 39
# speedup vs baseline: 1.2099x; 1.0428x over previous
"""Trainium2 Bass kernel for the dual cross-attention similarity module.

Math (per query q of 300, way w of 5):
  qkv from shared W; B->A attention (25 b-tokens over 125 a-tokens/way) and
  A->B attention (125 a-tokens/way over 25 b-tokens); outputs are negated
  squared Frobenius distances between v and softmax-reconstructions.

Sharding: queries split 40/core across 8 cores (300 padded to 320);
features_a / W / consts replicated; no collectives.

Per-core design (v3):
  * Inputs quantized host-side to fp8e4m3 (W pre-scaled x2; outputs
    unscaled /WS^2 in the final device ops) -> 1.3MB load.
  * qkv runs as fp8 DoubleRow matmuls over a 768-padded contraction.
  * q/k/v are retiled to [64, 2, T] fp8 via sbuf->sbuf DMAs so the score
    and cross-gram matmuls also run DoubleRow at 0.5 cyc/row (stationaries
    padded to even widths: a-ways at 128, b-groups via [64,2,8,128]).
  * n_a / n_b fold into Z-independent terms: n_b rides as row 96 of the
    B accumulator bank (through the same evac + transpose path as Z/C1/C2);
    n_a rides as row 96 of phase C's V bank.
  * B->A per 500-col chunk: 5 ways of DR scores in two psum tiles, fused
    exps; R = V_a E; Z/c1/c2 thin-reduced into one psum bank (rows
    0:5/32:37/64:69). Tail: rows transposed to [128 l, 8, 97] slots,
    rational math on 40-col tiles, indicator matmuls -> [40, 6].
  * A->B: way-aligned a-chunks (384, 256); per group: DR scores + DR
    cross-gram, aeb = blockdiag-gram @ E, Z/U/V accumulated via bo40;
    tail reads psum directly with way-aligned 125-slices.  The B tail is
    emitted between the two C chunks to overlap its serial latency.
"""

import numpy as np
import ml_dtypes

import concourse.bass as bass
import concourse.bacc as bacc
import concourse.tile as tile
from concourse import mybir
from concourse.bass_utils import run_bass_kernel_spmd

F32 = mybir.dt.float32
BF16 = mybir.dt.bfloat16
FP8 = mybir.dt.float8e4
AL = mybir.AluOpType
AF = mybir.ActivationFunctionType
AX = mybir.AxisListType
DR = mybir.MatmulPerfMode.DoubleRow

WS = 2.0                      # host-side W scale (outputs carry WS^2)
SCALE = 0.08838834764831845 / (WS * WS)   # 1/sqrt(128) / WS^2
N_CORES = 8
NQ = 40          # queries per core
LB = 25          # b tokens per query
LA = 125         # a tokens per way
NW = 5
TB = NQ * LB     # 1000
TAP = NW * 128   # 640 (a tokens padded to 128/way)

# consts blob (bf16) column offsets
C_ID = 0          # ident [128,128]
C_OW5 = 128       # [125,5,5]
C_OW128 = 153     # [128,5,5]
C_BO40 = 178      # [125,8,40]
C_BDM = 498       # [125,125]
C_ONES = 623      # [128,1]
C_ONES40 = 624    # [1,40]
NCB = 664
# f32 blob offsets
F_IND = 0         # [128,8,40]
F_ID = 320        # [128,128]
NCF = 448


def build_nc():
    nc = bacc.Bacc("TRN2", target_bir_lowering=False, debug=False)

    fa_d = nc.dram_tensor("fa", [640, TAP], FP8, kind="ExternalInput")
    fb_d = nc.dram_tensor("fb", [640, TB], FP8, kind="ExternalInput")
    wt_d = nc.dram_tensor("wt", [640, 384], FP8, kind="ExternalInput")
    cb_d = nc.dram_tensor("cb", [128, NCB], BF16, kind="ExternalInput")
    cf_d = nc.dram_tensor("cf", [128, NCF], F32, kind="ExternalInput")
    sq_d = nc.dram_tensor("sq", [NQ, NW], F32, kind="ExternalOutput")
    qs_d = nc.dram_tensor("qs", [NQ, NW], F32, kind="ExternalOutput")

    with tile.TileContext(nc) as tc:
        with (
            tc.tile_pool(name="const", bufs=1) as const,
            tc.tile_pool(name="feat", bufs=1) as feat,
            tc.tile_pool(name="persist", bufs=1) as persist,
            tc.tile_pool(name="ew", bufs=1) as ew,
            tc.tile_pool(name="work", bufs=2) as work,
        ):
            # ---------------- loads ----------------
            wt = feat.tile([128, 6, 384], FP8)
            nc.gpsimd.memset(wt[:, 5, :], 0.0)
            nc.sync.dma_start(out=wt[:, 0:5, :],
                              in_=wt_d.rearrange("(cb c) e -> c cb e", c=128))
            fa = feat.tile([128, 6, TAP], FP8)
            nc.gpsimd.memset(fa[:, 5, :], 0.0)
            nc.sync.dma_start(out=fa[:, 0:5, :],
                              in_=fa_d.rearrange("(cb c) t -> c cb t", c=128))
            fb = feat.tile([128, 6, TB], FP8)
            nc.gpsimd.memset(fb[:, 5, :], 0.0)
            nc.sync.dma_start(out=fb[:, 0:5, :],
                              in_=fb_d.rearrange("(cb c) t -> c cb t", c=128))
            cb = const.tile([128, NCB], BF16)
            nc.sync.dma_start(out=cb, in_=cb_d[:])
            cf = const.tile([128, NCF], F32)
            nc.sync.dma_start(out=cf, in_=cf_d[:])
            ident = cb[:, C_ID:C_ID + 128]
            ow5 = cb[0:125, C_OW5:C_OW5 + 25].rearrange("p (w c) -> p w c", w=5)
            ow128 = cb[:, C_OW128:C_OW128 + 25].rearrange("p (w c) -> p w c", w=5)
            bo40 = cb[0:125, C_BO40:C_BO40 + 320].rearrange("p (g c) -> p g c", g=8)
            bdm = cb[0:125, C_BDM:C_BDM + 125]
            ones128 = cb[:, C_ONES:C_ONES + 1]
            ones40 = cb[0:1, C_ONES40:C_ONES40 + 40]
            indB = cf[:, F_IND:F_IND + 320].rearrange("p (g c) -> p g c", g=8)
            identf = cf[:, F_ID:F_ID + 128]

            # warm ACT tables (exp/copy/square live in one set)
            warm = work.tile([1, 1], F32, tag="warm")
            nc.scalar.activation(out=warm, in_=cb[0:1, 0:1], func=AF.Exp)

            # ---------------- phase A: qkv ----------------
            qa8 = persist.tile([128, TAP], FP8)
            ka8 = persist.tile([128, TAP], FP8)
            vaB = persist.tile([128, TAP], BF16)
            va8 = persist.tile([128, TAP], FP8)
            qb8 = persist.tile([128, TB], FP8)
            kb8 = persist.tile([128, TB], FP8)
            vbB = persist.tile([128, TB], BF16)
            vb8 = persist.tile([128, TB], FP8)

            psA_cm = tc.tile_pool(name="psA", bufs=1, space="PSUM")
            psA = psA_cm.__enter__()

            def qkv_emit(src, chunks, dests):
                for e in range(3):
                    done = 0
                    for grp in chunks:
                        gw = sum(grp)
                        pq = psA.tile([128, 512], F32, tag="pqkv", bufs=3)
                        off = 0
                        for cw in grp:
                            c0 = done + off
                            for j in range(3):
                                nc.tensor.matmul(
                                    pq[:, off:off + cw],
                                    wt[:, 2 * j:2 * j + 2, e * 128:(e + 1) * 128],
                                    src[:, 2 * j:2 * j + 2, c0:c0 + cw],
                                    start=(j == 0), stop=(j == 2),
                                    perf_mode=DR,
                                )
                            off += cw
                        for k, dst in enumerate(dests[e]):
                            if not (e == 2 and k == 1):
                                nc.scalar.copy(out=dst[:, done:done + gw],
                                               in_=pq[:, 0:gw])
                            else:
                                nc.vector.tensor_copy(out=dst[:, done:done + gw],
                                                      in_=pq[:, 0:gw])
                        done += gw

            qkv_emit(fa, [[256, 256], [128]], [[qa8], [ka8], [vaB, va8]])
            # B-critical retile first
            kaD = persist.tile([64, 2, TAP], FP8)
            nc.sync.dma_start(out=kaD[:, 0, :], in_=ka8[0:64, :])
            nc.sync.dma_start(out=kaD[:, 1, :], in_=ka8[64:128, :])
            # vA in token-major via transposes (rp stationary)
            vA_tok = persist.tile([125, 5, 128], BF16)
            ptp = psA.tile([125, 5, 128], BF16, tag="ptp", bufs=1)
            for w in range(5):
                nc.tensor.transpose(ptp[:, w, :],
                                    vaB[:, w * 128:w * 128 + 125], ident)
            nc.vector.tensor_copy(out=vA_tok, in_=ptp)
            sqa = work.tile([128, TAP], BF16, tag="sqa")
            nc.gpsimd.tensor_mul(sqa, vaB, vaB)

            qkv_emit(fb, [[250, 250], [250, 250]], [[qb8], [kb8], [vbB, vb8]])
            qbD = persist.tile([64, 2, TB], FP8)
            nc.sync.dma_start(out=qbD[:, 0, :], in_=qb8[0:64, :])
            nc.sync.dma_start(out=qbD[:, 1, :], in_=qb8[64:128, :])
            qaD = persist.tile([64, 2, TAP], FP8)
            vaD = persist.tile([64, 2, TAP], FP8)
            for s8, dst in ((qa8, qaD), (va8, vaD)):
                nc.sync.dma_start(out=dst[:, 0, :], in_=s8[0:64, :])
                nc.sync.dma_start(out=dst[:, 1, :], in_=s8[64:128, :])
            kbD = persist.tile([64, 2, 8, 128], FP8)
            vbD = persist.tile([64, 2, 8, 128], FP8)
            for s8, dst in ((kb8, kbD), (vb8, vbD)):
                nc.gpsimd.memset(dst[:, :, :, 125:128], 0.0)
                nc.sync.dma_start(
                    out=dst[:, 0, :, 0:125],
                    in_=s8[0:64, :].rearrange("p (g t) -> p g t", g=8))
                nc.sync.dma_start(
                    out=dst[:, 1, :, 0:125],
                    in_=s8[64:128, :].rearrange("p (g t) -> p g t", g=8))
            sqb = work.tile([128, TB], BF16, tag="sqb")
            nc.vector.tensor_mul(sqb, vbB, vbB)
            pnb = psA.tile([1, 2, 512], F32, tag="pnb", bufs=1,
                           padded_shape=[128, 2, 512])
            nc.tensor.matmul(pnb[:, 0, 0:500], ones128, sqb[:, 0:500],
                             start=True, stop=True)
            nc.tensor.matmul(pnb[:, 1, 0:500], ones128, sqb[:, 500:1000],
                             start=True, stop=True)
            nbS = persist.tile([1, 1024], F32)
            nc.gpsimd.memset(nbS[:, TB:1024], 0.0)
            nc.scalar.copy(out=nbS[:, 0:TB].rearrange("p (two c) -> p two c",
                                                      two=2),
                           in_=pnb[:, :, 0:500])
            psA_cm.__exit__(None, None, None)

            # blockdiag gram of v_b per group (deep-buffered: no PE stall)
            ptr2_cm = tc.tile_pool(name="ptr2", bufs=1, space="PSUM")
            ptr2 = ptr2_cm.__enter__()
            bds = persist.tile([125, 8, 125], BF16)
            for g in range(8):
                gsl = slice(g * 125, (g + 1) * 125)
                pgram = ptr2.tile([125, 128], F32, tag="pgram", bufs=6)
                nc.tensor.matmul(pgram[:, 0:125], vb8[:, gsl], vb8[:, gsl],
                                 start=True, stop=True)
                nc.vector.tensor_mul(bds[:, g, :], pgram[:, 0:125], bdm)
            ptr2_cm.__exit__(None, None, None)

            # ---------------- phase B: B attends A ----------------
            psB_cm = tc.tile_pool(name="psB", bufs=1, space="PSUM")
            psB = psB_cm.__enter__()
            # zs rows: 0:5 Z, 32:37 C1, 64:69 C2
            zs = persist.tile([69, 1024], F32)
            nc.gpsimd.memset(zs[:, TB:1024], 1.0)
            sbaA = psB.tile([128, 2, 512], F32, tag="sbaA", bufs=1)
            sbaB = psB.tile([128, 3, 512], F32, tag="sbaB", bufs=1)
            zcc = psB.tile([128, 512], F32, tag="zcc", bufs=1)
            for ci in range(2):
                c0 = ci * 500
                for w in range(5):
                    dst = sbaA[:, w, :] if w < 2 else sbaB[:, w - 2, :]
                    for h in range(2):
                        nc.tensor.matmul(
                            dst[:, h * 250:(h + 1) * 250],
                            kaD[:, :, w * 128:(w + 1) * 128],
                            qbD[:, :, c0 + h * 250:c0 + (h + 1) * 250],
                            start=True, stop=True, perf_mode=DR)
                eA = ew.tile([128, 2, 500], BF16, tag="eA", bufs=1)
                nc.scalar.activation(out=eA, in_=sbaA[:, :, 0:500],
                                     func=AF.Exp, scale=SCALE)
                eB = ew.tile([128, 3, 500], BF16, tag="eB", bufs=1)
                nc.scalar.activation(out=eB, in_=sbaB[:, :, 0:500],
                                     func=AF.Exp, scale=SCALE)
                for w in range(5):
                    e_w = (eA[:, w, :] if w < 2 else eB[:, w - 2, :])[0:125, :]
                    rp = psB.tile([128, 512], F32, tag="rp", bufs=2)
                    nc.tensor.matmul(rp[:, 0:500], vA_tok[:, w, :], e_w,
                                     start=True, stop=True)
                    c1sb = work.tile([128, 500], BF16, tag="c1sb", bufs=2)
                    nc.vector.tensor_mul(c1sb, rp[:, 0:500],
                                         vbB[:, c0:c0 + 500])
                    c2sb = work.tile([128, 500], BF16, tag="c2sb", bufs=2)
                    nc.scalar.activation(out=c2sb, in_=rp[:, 0:500],
                                         func=AF.Square)
                    nc.tensor.matmul(zcc[0:5, 0:500], ow5[:, w, :], e_w,
                                     start=(w == 0), stop=(w == 4))
                    nc.tensor.matmul(zcc[32:37, 0:500], ow128[:, w, :], c1sb,
                                     start=(w == 0), stop=(w == 4))
                    nc.tensor.matmul(zcc[64:69, 0:500], ow128[:, w, :], c2sb,
                                     start=(w == 0), stop=(w == 4))
                nc.scalar.copy(out=zs[0:69, c0:c0 + 500], in_=zcc[0:69, 0:500])
            psB_cm.__exit__(None, None, None)

            # ---------------- phase C (+ B tail interleaved) ----------------
            psC_cm = tc.tile_pool(name="psC", bufs=1, space="PSUM")
            psC = psC_cm.__enter__()
            psT_cm = tc.tile_pool(name="psT", bufs=1, space="PSUM")
            psT = psT_cm.__enter__()

            misc = psT.tile([128, 512], F32, tag="misc", bufs=1)

            def emit_b_tail():
                # zs -> [128 l, 8 slice, 97] transposed slots
                TtS = persist.tile([128, 8, 70], F32)
                for half in range(2):
                    Tt = psT.tile([128, 4, 128], F32, tag="Tt", bufs=1)
                    for sl in range(4):
                        s = half * 4 + sl
                        nc.tensor.transpose(Tt[:, sl, 0:69],
                                            zs[:, s * 128:(s + 1) * 128],
                                            identf[0:69, 0:69])
                        nc.tensor.transpose(Tt[:, sl, 69:70],
                                            nbS[:, s * 128:(s + 1) * 128],
                                            identf[0:1, 0:1])
                    nc.scalar.copy(out=TtS[:, half * 4:half * 4 + 4, 0:70],
                                   in_=Tt[:, :, 0:70])
                rT = work.tile([128, 8, 5], F32, tag="rT")
                nc.vector.reciprocal(out=rT, in_=TtS[:, :, 0:5])
                u1T = work.tile([128, 8, 5], F32, tag="u1T")
                nc.gpsimd.tensor_mul(u1T, TtS[:, :, 64:69], rT)
                t3 = work.tile([128, 8, 5], F32, tag="t3")
                nc.vector.scalar_tensor_tensor(out=t3, in0=TtS[:, :, 32:37],
                                               scalar=-2.0, in1=u1T,
                                               op0=AL.mult, op1=AL.add)
                nc.gpsimd.tensor_mul(TtS[:, :, 64:69], t3, rT)
                qsp = misc[0:40, 0:6]
                for s in range(8):
                    nc.tensor.matmul(qsp, indB[:, s, :], TtS[:, s, 64:70],
                                     start=(s == 0), stop=(s == 7))
                qsb = work.tile([40, 6], F32, tag="qsb")
                nc.vector.tensor_copy(out=qsb, in_=qsp)
                qs_sb = work.tile([40, 5], F32, tag="qs_sb")
                nc.vector.tensor_scalar(
                    out=qs_sb, in0=qsb[:, 0:5], scalar1=qsb[:, 5:6],
                    scalar2=-1.0 / (WS * WS), op0=AL.add, op1=AL.mult)
                nc.sync.dma_start(out=qs_d[:], in_=qs_sb)

            sq_parts = work.tile([40, 5], F32, tag="sq_parts")
            naB = persist.tile([1, 5], BF16)
            napb_ap = None
            mul_ctr = [0]
            for ci, (c0, cn) in enumerate(((0, 384), (384, 256))):
                nwc = cn // 128
                zuv = psC.tile([128, 512], F32, tag="zuv", bufs=1)
                vp = psC.tile([128, 512], F32, tag="vp", bufs=1)
                # zuv rows: 0:40 Z, 64:104 U ; vp rows: 0:40 V, 64:104 na-b,
                # 96 (via tile_position) n_a row
                for gp in range(4):
                    sab = psC.tile([128, 2, 512], F32, tag="sab", bufs=1)
                    for gg in range(2):
                        g = gp * 2 + gg
                        for h in range(2):
                            h0, hn = h * (cn // 2), cn // 2
                            nc.tensor.matmul(
                                sab[:, gg, h0:h0 + hn],
                                kbD[:, :, g, :],
                                qaD[:, :, c0 + h0:c0 + h0 + hn],
                                start=True, stop=True, perf_mode=DR)
                    eg2 = ew.tile([128, 2, 384], BF16, tag="eg2", bufs=2)
                    nc.scalar.activation(out=eg2[:, :, 0:cn],
                                         in_=sab[:, :, 0:cn],
                                         func=AF.Exp, scale=SCALE)
                    for gg in range(2):
                        g = gp * 2 + gg
                        e_g = eg2[0:125, gg, 0:cn]
                        gab = psC.tile([128, 512], F32, tag="gab", bufs=2)
                        for h in range(2):
                            h0, hn = h * (cn // 2), cn // 2
                            nc.tensor.matmul(
                                gab[:, h0:h0 + hn], vbD[:, :, g, :],
                                vaD[:, :, c0 + h0:c0 + h0 + hn],
                                start=True, stop=True, perf_mode=DR)
                        egp = work.tile([125, 384], BF16, tag="egp", bufs=2)
                        mul_ctr[0] += 1
                        if mul_ctr[0] % 3 == 0:
                            # shed DVE: evac via Act, multiply on Pool
                            gabS = work.tile([125, 384], BF16, tag="gabS",
                                             bufs=2)
                            nc.scalar.copy(out=gabS[:, 0:cn],
                                           in_=gab[0:125, 0:cn])
                            nc.gpsimd.tensor_mul(egp[:, 0:cn], e_g,
                                                 gabS[:, 0:cn])
                        else:
                            nc.vector.tensor_mul(egp[:, 0:cn], e_g,
                                                 gab[0:125, 0:cn])
                        aeb = psC.tile([128, 512], F32, tag="gab", bufs=2)
                        nc.tensor.matmul(aeb[0:125, 0:cn], bds[:, g, :], e_g,
                                         start=True, stop=True)
                        eab = work.tile([125, 384], BF16, tag="eab", bufs=2)
                        nc.vector.tensor_mul(eab[:, 0:cn], e_g,
                                             aeb[0:125, 0:cn])
                        nc.tensor.matmul(zuv[0:40, 0:cn], bo40[:, g, :], e_g,
                                         start=(g == 0), stop=(g == 7))
                        nc.tensor.matmul(zuv[64:104, 0:cn], bo40[:, g, :],
                                         egp[:, 0:cn],
                                         start=(g == 0), stop=(g == 7))
                        nc.tensor.matmul(vp[0:40, 0:cn], bo40[:, g, :],
                                         eab[:, 0:cn],
                                         start=(g == 0), stop=(g == 7))
                # n_a row for this chunk + per-way reduce
                na_row = misc[64:65, 0:cn]
                nc.tensor.matmul(na_row, ones128,
                                 sqa[:, c0:c0 + cn], start=True, stop=True)
                naF = work.tile([1, 5], F32, tag="naF", bufs=2)
                nc.vector.tensor_reduce(
                    out=naF[:, 0:nwc],
                    in_=na_row.rearrange("p (w l) -> p w l", w=nwc),
                    op=AL.add, axis=AX.X)
                nc.scalar.copy(out=naB[:, ci * 3:ci * 3 + nwc],
                               in_=naF[:, 0:nwc])
                if ci == 1:
                    napb_ap = misc[0:40, 8:13]
                    nc.tensor.matmul(napb_ap, ones40, naB,
                                     start=True, stop=True)
                # tail: f = (2U - V r) r summed over each way's 125 cols
                rab = work.tile([40, 384], F32, tag="rab")
                nc.vector.reciprocal(out=rab[:, 0:cn], in_=zuv[0:40, 0:cn])
                u1 = work.tile([40, 384], F32, tag="u1")
                nc.vector.tensor_mul(u1[:, 0:cn], vp[0:40, 0:cn],
                                     rab[:, 0:cn])
                u2 = work.tile([40, 384], F32, tag="u2")
                nc.vector.scalar_tensor_tensor(
                    out=u2[:, 0:cn], in0=zuv[64:104, 0:cn], scalar=2.0,
                    in1=u1[:, 0:cn], op0=AL.mult, op1=AL.subtract)
                f2 = work.tile([40, 384], F32, tag="f2")
                nc.gpsimd.tensor_mul(f2[:, 0:cn], u2[:, 0:cn], rab[:, 0:cn])
                nc.vector.tensor_reduce(
                    out=sq_parts[:, ci * 3:ci * 3 + nwc],
                    in_=f2[:, 0:cn].rearrange(
                        "p (w l) -> p w l", w=nwc)[:, :, 0:125],
                    op=AL.add, axis=AX.X)
                if ci == 0:
                    emit_b_tail()
            sqt = work.tile([40, 5], F32, tag="sqt")
            nc.vector.tensor_sub(sqt, sq_parts, napb_ap)
            sq_sb = work.tile([40, 5], F32, tag="sq_sb")
            nc.scalar.activation(out=sq_sb, in_=sqt, func=AF.Copy,
                                 scale=1.0 / (WS * WS))
            nc.sync.dma_start(out=sq_d[:], in_=sq_sb)
            psT_cm.__exit__(None, None, None)
            psC_cm.__exit__(None, None, None)

    nc.compile()
    return nc


_CACHE = {}


def _get_nc():
    if "nc" not in _CACHE:
        _CACHE["nc"] = build_nc()
    return _CACHE["nc"]


def _consts():
    cb = np.zeros((128, NCB), np.float32)
    cb[:, C_ID:C_ID + 128] = np.eye(128)
    ow5 = np.zeros((125, 5, 5), np.float32)
    ow128 = np.zeros((128, 5, 5), np.float32)
    for w in range(5):
        ow5[:, w, w] = 1.0
        ow128[:, w, w] = 1.0
    cb[0:125, C_OW5:C_OW5 + 25] = ow5.reshape(125, 25)
    cb[:, C_OW128:C_OW128 + 25] = ow128.reshape(128, 25)
    bo125 = np.kron(np.eye(5, dtype=np.float32), np.ones((25, 1), np.float32))
    bo40 = np.zeros((125, 8, 40), np.float32)
    for g in range(8):
        bo40[:, g, 5 * g:5 * g + 5] = bo125
    cb[0:125, C_BO40:C_BO40 + 320] = bo40.reshape(125, 320)
    cb[0:125, C_BDM:C_BDM + 125] = np.kron(
        np.eye(5, dtype=np.float32), np.ones((25, 25), np.float32))
    cb[:, C_ONES] = 1.0
    cb[0, C_ONES40:C_ONES40 + 40] = 1.0

    cf = np.zeros((128, NCF), np.float32)
    indB = np.zeros((128, 8, 40), np.float32)
    for s in range(8):
        for p in range(128):
            l = s * 128 + p
            if l < TB:
                indB[p, s, l // 25] = 1.0
    cf[:, F_IND:F_IND + 320] = indB.reshape(128, 320)
    cf[:, F_ID:F_ID + 128] = np.eye(128)
    return (cb.astype(ml_dtypes.bfloat16), cf)


def kernel(features_a, features_b, W):
    features_a = np.asarray(features_a, np.float32)
    features_b = np.asarray(features_b, np.float32)
    W = np.asarray(W, np.float32)
    f8 = ml_dtypes.float8_e4m3

    nq_total = features_b.shape[0]
    fbp = np.zeros((N_CORES * NQ, 640, LB), np.float32)
    fbp[:nq_total] = features_b
    fb_t = np.ascontiguousarray(fbp.transpose(1, 0, 2)).astype(f8)
    fa_pad = np.zeros((640, NW, 128), np.float32)
    fa_pad[:, :, :LA] = features_a.transpose(1, 0, 2)
    fa_t = np.ascontiguousarray(fa_pad.reshape(640, TAP)).astype(f8)
    wt = np.ascontiguousarray(W.T * WS).astype(f8)
    cb, cf = _consts()

    in_maps = []
    for c in range(N_CORES):
        m = {
            "fa": fa_t,
            "fb": np.ascontiguousarray(
                fb_t[:, c * NQ:(c + 1) * NQ, :]).reshape(640, TB),
            "wt": wt,
            "cb": cb,
            "cf": cf,
        }
        in_maps.append(m)

    nc = _get_nc()
    res = run_bass_kernel_spmd(nc, in_maps, core_ids=list(range(N_CORES)))

    sq = np.zeros((N_CORES * NQ, NW), np.float32)
    qs = np.zeros((N_CORES * NQ, NW), np.float32)
    for c in range(N_CORES):
        sq[c * NQ:(c + 1) * NQ] = res.results[c]["sq"]
        qs[c * NQ:(c + 1) * NQ] = res.results[c]["qs"]
    return sq[:nq_total], qs[:nq_total]
